# revision 26
# baseline (speedup 1.0000x reference)
"""Trainium2 Bass kernel for nn_AgentGnn (2-layer CGConv GNN, 128 scenes x 64 agents).

Structure exploited:
- Edges are fully-connected per 64-agent scene (no self loops), so gather/scatter
  becomes dense 64x64 blocks: agg[i] = sum_j sigmoid(F_ij) * softplus(S_ij) - diag.
- Per-edge linear terms factor into per-node terms:
    F_ij = af[i] + bf[j],  af = x_i @ Wf[:D] + c_i @ Wf[2D:] (+bias via ACT),
                           bf = x_j @ Wf[D:2D] - c_j @ Wf[2D:]
- Pairwise sums F[d,(i,j)] are built on TensorE with a 0/1 indicator matmul
  (indicator generated on device via affine_select) against a stacked
  [af_scene; bf_scene] stationary operand.
- softplus = ln(1+exp(.)) (Exp+Ln share one ACT table set); sigmoid in another
  set; all 16 scenes batched per set so tables load twice per layer.
- BatchNorm stats are global over all 8192 nodes -> tiny [128,2] AllReduce/layer,
  computed in two fused reductions over the full agg tile at layer end.
- All inputs packed into ONE [128, 1560] f32 tensor per core (x | biases |
  bitcast-f16 node weights | packed centers+edge weights) to minimize
  per-dispatch argument overhead. Sharding: 16 scenes per core, data parallel.
"""

import numpy as np

N_SAMPLES = 128
AGENTS = 64
D = 128
EDIM = 2
N = N_SAMPLES * AGENTS
EPS = 1e-5

N_CORES = 8
SCENES_PC = N_SAMPLES // N_CORES      # 16 scenes per core
NODES_PC = SCENES_PC * AGENTS         # 1024 nodes per core
PAIR = AGENTS * AGENTS                # 4096 pairwise cols per scene
CHUNK = 2048                          # pairwise chunk (32 i x 64 j)
N_CHUNKS = PAIR // CHUNK              # 2
I_PER_CHUNK = CHUNK // AGENTS         # 32
SUB = 8                               # scenes per sigmoid sub-batch

# packed input column layout (f32 columns)
XCOLS = NODES_PC          # 0:1024      x.T (f32)
BIAS0 = XCOLS             # 1024:1032   bf1,bs1,ga1,be1,bf2,bs2,ga2,be2
WN0 = BIAS0 + 8           # 1032:1544   node weights, f16 bitcast (8 x [D,128])
WN_BLK = D // 2           # 64 f32 cols per [D,128] f16 block
EC0 = WN0 + 8 * WN_BLK    # 1544:1560   centers.T + edge weights, f16 packed
EC_F16 = EDIM * (NODES_PC + 8 * D)    # 2 x (1024 + 1024) f16 values
EC_COLS = EC_F16 // 2 // D            # 16 f32 cols
IDEN0 = EC0 + EC_COLS     # 1560:1592   identity64 f16 seed (rows 0:64)
TOTC = IDEN0 + 32         # 1592

_CACHE: dict = {}


def _expected_edges():
    a = np.arange(AGENTS)
    rows = np.repeat(a, AGENTS)
    cols = np.tile(a, AGENTS)
    mask = rows != cols
    rows, cols = rows[mask], cols[mask]
    offsets = (np.arange(N_SAMPLES) * AGENTS)[:, None]
    src = (rows[None, :] + offsets).ravel().astype(np.int32)
    dst = (cols[None, :] + offsets).ravel().astype(np.int32)
    return src, dst


def _numpy_fallback(gnn_in, centers, edge_src, edge_dst, ws):
    # generic (slow) reference path, used only if edges don't match the
    # expected block-diagonal fully-connected pattern
    def sigmoid(x):
        return 1.0 / (1.0 + np.exp(-x))

    def softplus(x):
        return np.logaddexp(0.0, x)

    x = gnn_in.astype(np.float64)
    e = (centers[edge_dst] - centers[edge_src]).astype(np.float64)
    for li in (1, 2):
        Wf, bf, Ws, bs, gamma, beta = (ws[f"Wf{li}"], ws[f"bf{li}"], ws[f"Ws{li}"],
                                       ws[f"bs{li}"], ws[f"gamma{li}"], ws[f"beta{li}"])
        z = np.concatenate([x[edge_dst], x[edge_src], e], axis=-1)
        m = sigmoid(z @ Wf + bf) * softplus(z @ Ws + bs)
        agg = np.zeros((N, D))
        np.add.at(agg, edge_dst, m)
        mu = agg.mean(axis=0)
        var = agg.var(axis=0)
        agg = (agg - mu) / np.sqrt(var + EPS) * gamma + beta
        x = np.maximum(agg + x, 0.0)
    return x.astype(np.float32)


def _build_nc(use_collectives=True, unroll=1, dup_act=1, dup_vec=1, dup_mm=1, old_tree=False, exp_sbuf=True, scene_tree=False):
    import concourse.bacc as bacc
    import concourse.mybir as mybir
    import concourse.tile as tile
    from concourse.tile_rust import add_dep_helper

    f32 = mybir.dt.float32
    f16 = mybir.dt.float16
    bf16 = mybir.dt.bfloat16
    AF = mybir.ActivationFunctionType
    OP = mybir.AluOpType

    nc = bacc.Bacc("TRN2", target_bir_lowering=False, debug=False,
                   num_devices=N_CORES if use_collectives else 1)

    # ---- I/O: single packed input ----
    xt_in = nc.dram_tensor("xt", [D, TOTC], f32, kind="ExternalInput")
    out_t = nc.dram_tensor("out_t", [D, NODES_PC], f32, kind="ExternalOutput")

    acts = []  # ACT instructions in intended engine order

    def act(*args, **kwargs):
        for _ in range(dup_act):
            inst = nc.scalar.activation(*args, **kwargs)
            acts.append(inst)
        return inst

    def vtt(*args, **kwargs):
        for _ in range(dup_vec):
            inst = nc.vector.tensor_tensor(*args, **kwargs)
        return inst

    def mm(*args, **kwargs):
        for _ in range(dup_mm):
            inst = nc.tensor.matmul(*args, **kwargs)
        return inst

    with tile.TileContext(nc) as tc:
        with (
            tc.tile_pool(name="cst", bufs=1) as cst,
            tc.tile_pool(name="wrk", bufs=1) as wrk,
            tc.tile_pool(name="chk", bufs=3) as chk,
            tc.tile_pool(name="epool", bufs=4) as epool,
            tc.tile_pool(name="ps", bufs=2, space="PSUM") as ps,
            tc.tile_pool(name="dram", bufs=1, space="DRAM") as dram,
        ):
            # ---- load + unpack the single input ----
            xt = cst.tile([D, TOTC], f32)
            nc.sync.dma_start(xt[:], xt_in.ap())
            ec = cst.tile([EDIM, EC_F16 // EDIM], f16)   # [2, 2048]
            nc.sync.dma_start(ec[:], xt[:, EC0:IDEN0].bitcast(f16))

            wn16 = cst.tile([D, 8 * D], f16)
            nc.vector.tensor_copy(
                wn16[:], xt[:, WN0:WN0 + 8 * WN_BLK].bitcast(f16))

            def wblk(idx):
                return wn16[:, idx * D:(idx + 1) * D]      # [D, 128] f16

            def eblk(idx):
                c0 = NODES_PC + idx * D
                return ec[:, c0:c0 + D]                    # [2, 128] f16

            def bias(idx):
                return xt[:, BIAS0 + idx:BIAS0 + idx + 1]  # [D, 1] f32

            # ---- indicator built on device from the identity seed ----
            iden = cst.tile([AGENTS, AGENTS], f16)
            nc.sync.dma_start(iden[:], xt[0:AGENTS, IDEN0:TOTC].bitcast(f16))
            ind = cst.tile([128, PAIR], f16)
            nc.vector.tensor_copy(
                ind[0:64, :].rearrange("p (a b) -> p a b", b=AGENTS),
                iden[:].rearrange("p (a b) -> p a b", b=1).broadcast_to(
                    [AGENTS, AGENTS, AGENTS]))
            nc.vector.tensor_copy(
                ind[64:128, :].rearrange("p (a b) -> p a b", b=AGENTS),
                iden[:].rearrange("p (a b) -> p a b", a=1).broadcast_to(
                    [AGENTS, AGENTS, AGENTS]))

            x_cur = xt[:, 0:NODES_PC]
            for rep in range(unroll):
              for li0 in (1, 2):
                li = f"{li0}" if rep == 0 else f"{li0}r{rep}"
                w_off = (li0 - 1) * 4
                wfd, wfs = wblk(w_off + 0), wblk(w_off + 1)
                wsd, wss = wblk(w_off + 2), wblk(w_off + 3)
                wfe, wfen = eblk(w_off + 0), eblk(w_off + 1)
                wse, wsen = eblk(w_off + 2), eblk(w_off + 3)
                bf, bs = bias((li0 - 1) * 4 + 0), bias((li0 - 1) * 4 + 1)
                ga, be = bias((li0 - 1) * 4 + 2), bias((li0 - 1) * 4 + 3)

                # fp16 copy of x for the node matmuls (residual stays fp32)
                x16 = wrk.tile([D, NODES_PC], f16, name=f"x16_{li}", tag="x16")
                nc.vector.tensor_copy(x16[:], x_cur)

                agg = wrk.tile([D, NODES_PC], f32, name=f"agg{li}", tag="agg")

                abf, abs_ = {}, {}

                def node_phase(batch, bi):
                    # one PSUM slab per batch: F-path node terms [af;bf] in
                    # cols 0:1024 (node-major, feeds the indicator matmul) and
                    # S-path terms TRANSPOSED (d-major: as 1024:1536, bs
                    # 1536:2048) so exp factorizes per node and two batched
                    # Exp ACTs replace the full-size pairwise Exp pass.
                    nb = len(batch)
                    slab = ps.tile([128, 4 * AGENTS * nb], f32,
                                   name=f"slab{bi}_{li}", tag="pp")
                    a0 = 2 * AGENTS * nb          # as-region start (1024)
                    b0 = 3 * AGENTS * nb          # bs-region start (1536)
                    for k, s in enumerate(batch):
                        xs = x16[:, s * AGENTS:(s + 1) * AGENTS]
                        cs = ec[:, s * AGENTS:(s + 1) * AGENTS]
                        pab = slab[:, k * D:(k + 1) * D]
                        nc.tensor.matmul(pab[0:64, :], lhsT=cs, rhs=wfe, start=True, stop=False)
                        nc.tensor.matmul(pab[0:64, :], lhsT=xs, rhs=wfd, start=False, stop=True)
                        nc.tensor.matmul(pab[64:128, :], lhsT=cs, rhs=wfen, start=True, stop=False)
                        nc.tensor.matmul(pab[64:128, :], lhsT=xs, rhs=wfs, start=False, stop=True)
                        pas = slab[:, a0 + k * AGENTS:a0 + (k + 1) * AGENTS]
                        nc.tensor.matmul(pas, lhsT=wse, rhs=cs, start=True, stop=False)
                        nc.tensor.matmul(pas, lhsT=wsd, rhs=xs, start=False, stop=True)
                        pbs = slab[:, b0 + k * AGENTS:b0 + (k + 1) * AGENTS]
                        nc.tensor.matmul(pbs, lhsT=wsen, rhs=cs, start=True, stop=False)
                        nc.tensor.matmul(pbs, lhsT=wss, rhs=xs, start=False, stop=True)
                    abt = wrk.tile([128, 2 * AGENTS * nb], f16,
                                   name=f"abt{bi}_{li}", tag=f"abt{bi}")
                    nc.vector.tensor_copy(abt[:], slab[:, 0:a0])
                    es = wrk.tile([D, 2 * AGENTS * nb], bf16,
                                  name=f"es{bi}_{li}", tag=f"es{bi}")
                    act(es[:, 0:AGENTS * nb], slab[:, a0:b0], AF.Exp,
                        bias=bs, scale=1.0)
                    act(es[:, AGENTS * nb:], slab[:, b0:], AF.Exp,
                        bias=0.0, scale=1.0)
                    for k, s in enumerate(batch):
                        abf[s] = abt[:, k * D:(k + 1) * D]
                        abs_[s] = (es[:, k * AGENTS:(k + 1) * AGENTS],
                                   es[:, (nb + k) * AGENTS:(nb + k + 1) * AGENTS])

                def phase_a(batch, sgs, Es):
                    for s in batch:
                        # prebuild E = ea (x) eb on DVE during the sigma window
                        # (DVE is otherwise idle here; keeps phase_b's Ln ACTs
                        # from stalling behind the outer products)
                        ea, eb = abs_[s]
                        E = epool.tile([D, PAIR], bf16, name=f"E{s % 4}", tag="E")
                        Es[s] = E
                        vtt(E[:].rearrange("p (a b) -> p a b", b=AGENTS),
                            ea.rearrange("p (a b) -> p a b", b=1).broadcast_to(
                                [D, AGENTS, AGENTS]),
                            eb.rearrange("p (a b) -> p a b", a=1).broadcast_to(
                                [D, AGENTS, AGENTS]),
                            OP.mult)
                        sg = wrk.tile([D, PAIR], f16, name=f"sg{s % SUB}",
                                      tag=f"sg{s % SUB}")
                        sgs[s] = sg
                        for c in range(N_CHUNKS):
                            pf = ps.tile([D, CHUNK], f32, name=f"pf{s}_{c}", tag="pp")
                            for k in range(CHUNK // 512):
                                col = c * CHUNK + k * 512
                                mm(pf[:, k * 512:(k + 1) * 512],
                                   lhsT=abf[s],
                                   rhs=ind[:, col:col + 512],
                                   start=True, stop=True)
                            act(sg[:, c * CHUNK:(c + 1) * CHUNK], pf[:],
                                AF.Sigmoid, bias=bf, scale=1.0)
                        # zero sigma on the diagonal -> m_ii = 0, so the
                        # j-reduction needs no diagonal correction
                        nc.vector.memset(
                            sg[:, 0:(AGENTS - 1) * (AGENTS + 1) + 1:AGENTS + 1],
                            0.0)

                def phase_b(batch, sgs, Es):
                    for s in batch:
                        E = Es[s]
                        pch = chk.tile([D, PAIR], f16, name="pch", tag="pch")
                        act(pch[:], E[:], AF.Ln, bias=1.0, scale=1.0)
                        uch = chk.tile([D, PAIR], f16, name="uch", tag="uch")
                        vtt(uch[:], sgs[s][:], pch[:], OP.mult)
                        for c in range(N_CHUNKS):
                            cur = uch[:, c * CHUNK:(c + 1) * CHUNK]
                            w = AGENTS
                            while w > 16:
                                h = w // 2
                                c3 = cur.rearrange("p (i j) -> p i j", j=w)
                                nxt = chk.tile([D, I_PER_CHUNK * h], f16,
                                               name=f"trc{h}", tag=f"trc{h}")
                                n3 = nxt.rearrange("p (i j) -> p i j", j=h)
                                vtt(n3, c3[:, :, 0:h], c3[:, :, h:w], OP.add)
                                cur = nxt[:]
                                w = h
                            off = s * AGENTS + c * I_PER_CHUNK
                            nc.vector.tensor_reduce(
                                agg[:, off:off + I_PER_CHUNK],
                                cur.rearrange("p (i j) -> p i j", j=w),
                                axis=mybir.AxisListType.X, op=OP.add)

                batches = [list(range(b0, min(b0 + SUB, SCENES_PC)))
                           for b0 in range(0, SCENES_PC, SUB)]
                sgs_all = [dict() for _ in batches]
                es_all = [dict() for _ in batches]
                node_phase(batches[0], 0)
                for bi, batch in enumerate(batches):
                    phase_a(batch, sgs_all[bi], es_all[bi])
                    if bi + 1 < len(batches):
                        node_phase(batches[bi + 1], bi + 1)
                    phase_b(batch, sgs_all[bi], es_all[bi])

                # ---- BN stats over the full agg tile (sum, sumsq) ----
                stats = wrk.tile([D, 2], f32, name=f"stats{li}", tag="stats")
                sq = wrk.tile([D, NODES_PC], f32, name="sq", tag="sg0")
                nc.vector.tensor_reduce(stats[:, 0:1], agg[:],
                                        axis=mybir.AxisListType.X, op=OP.add)
                nc.vector.tensor_tensor(sq[:], agg[:], agg[:], OP.mult)
                nc.vector.tensor_reduce(stats[:, 1:2], sq[:],
                                        axis=mybir.AxisListType.X, op=OP.add)

                # ---- AllReduce stats across cores ----
                cc_in = dram.tile([D, 2], f32, name=f"ccin{li}", tag=f"ccin{li}")
                cc_out = dram.tile([D, 2], f32, name=f"ccout{li}", tag=f"ccout{li}",
                                   addr_space="Shared")
                nc.sync.dma_start(cc_in[:], stats[:])
                if use_collectives:
                    nc.gpsimd.collective_compute(
                        "AllReduce", OP.add,
                        replica_groups=[list(range(N_CORES))],
                        ins=[cc_in.opt()], outs=[cc_out.opt()])
                else:
                    nc.sync.dma_start(cc_out[:], cc_in[:])
                stot = wrk.tile([D, 2], f32, name=f"stot{li}", tag="stot")
                nc.sync.dma_start(stot[:], cc_out[:])

                # mu, var, rstd = exp(-0.5*ln(var+eps)); A = gamma*rstd; B = beta-mu*A
                mu = wrk.tile([D, 1], f32, name="mu", tag="mu")
                ex2 = wrk.tile([D, 1], f32, name="ex2", tag="ex2")
                nc.vector.tensor_scalar_mul(mu[:], stot[:, 0:1], 1.0 / N)
                nc.vector.tensor_scalar_mul(ex2[:], stot[:, 1:2], 1.0 / N)
                var = wrk.tile([D, 1], f32, name="var", tag="var")
                nc.vector.tensor_tensor(var[:], mu[:], mu[:], OP.mult)
                nc.vector.tensor_tensor(var[:], ex2[:], var[:], OP.subtract)
                rstd = wrk.tile([D, 1], f32, name="rstd", tag="rstd")
                nc.vector.tensor_scalar_add(var[:], var[:], EPS)
                act(rstd[:], var[:], AF.Ln, bias=0.0, scale=1.0)
                act(rstd[:], rstd[:], AF.Exp, bias=0.0, scale=-0.5)
                A = wrk.tile([D, 1], f32, name="A", tag="A")
                Bt = wrk.tile([D, 1], f32, name="Bt", tag="Bt")
                nc.vector.tensor_tensor(A[:], ga, rstd[:], OP.mult)
                nc.vector.tensor_tensor(Bt[:], mu[:], A[:], OP.mult)
                nc.vector.tensor_tensor(Bt[:], be, Bt[:], OP.subtract)

                # x_next = relu(agg*A + B + x_cur), fused as STT + TS
                xn = wrk.tile([D, NODES_PC], f32, name=f"x{li}", tag=f"x{li0}")
                nc.vector.scalar_tensor_tensor(xn[:], agg[:], A[:, 0:1], x_cur,
                                               OP.mult, OP.add)
                nc.vector.tensor_scalar(xn[:], xn[:], Bt[:, 0:1], 0.0,
                                        OP.add, OP.max)
                x_cur = xn[:]

            nc.sync.dma_start(out_t.ap(), x_cur)

        # enforce ACT program order so table loads stay batched
        for a, b in zip(acts, acts[1:]):
            add_dep_helper(b.ins, a.ins, sync=False,
                           reason="ACT table-set batching order")

    # Restrict the act-table chooser to the two sets we actually want so
    # Exp and Ln resolve to the shared natural_log_exp set (the default
    # chooser alternates exp_and_others / natural_log, thrashing ~2.7us
    # table loads between every Exp and Ln).
    keep = {"sigmoid_and_others", "natural_log_exp_and_others"}
    orig_tables = bacc.get_activation_tables

    def patched_tables(arch):
        return {k: (v if k in keep else set())
                for k, v in orig_tables(arch).items()}

    bacc.get_activation_tables = patched_tables
    try:
        nc.compile()
    finally:
        bacc.get_activation_tables = orig_tables
    return nc


def _get_nc():
    if "nc" not in _CACHE:
        _CACHE["nc"] = _build_nc()
    return _CACHE["nc"]


def kernel(**inputs) -> np.ndarray:
    gnn_in = np.ascontiguousarray(np.asarray(inputs["gnn_in"], dtype=np.float32))
    centers = np.ascontiguousarray(np.asarray(inputs["centers"], dtype=np.float32))
    edge_src = np.asarray(inputs["edge_src"], dtype=np.int32)
    edge_dst = np.asarray(inputs["edge_dst"], dtype=np.int32)

    exp_src, exp_dst = _expected_edges()
    if not (np.array_equal(edge_src, exp_src) and np.array_equal(edge_dst, exp_dst)):
        return _numpy_fallback(
            gnn_in, centers, edge_src, edge_dst,
            {k: np.asarray(v, np.float32) for k, v in inputs.items()
             if k not in ("gnn_in", "centers", "edge_src", "edge_dst")})

    in_maps = _make_in_maps(inputs)
    run = _get_runner()
    per_core = run(in_maps)                      # [N_CORES, D, NODES_PC]
    out = np.concatenate(list(per_core), axis=1)  # [D, N]
    return np.ascontiguousarray(out.T)


def _get_runner():
    """Compile once; repeat kernel() calls reuse the jitted executable."""
    if "run" in _CACHE:
        return _CACHE["run"]
    import jax
    from jax.experimental.shard_map import shard_map
    from jax.sharding import Mesh, PartitionSpec

    import concourse.mybir as mybir
    from concourse import bass2jax

    bass2jax.install_neuronx_cc_hook()
    nc = _get_nc()
    partition_name = nc.partition_id_tensor.name if nc.partition_id_tensor else None
    in_names, out_names, out_avals = [], [], []
    for alloc in nc.m.functions[0].allocations:
        if not isinstance(alloc, mybir.MemoryLocationSet):
            continue
        name = alloc.memorylocations[0].name
        if alloc.kind == "ExternalInput":
            if name != partition_name:
                in_names.append(name)
        elif alloc.kind == "ExternalOutput":
            out_names.append(name)
            out_avals.append(jax.core.ShapedArray(
                tuple(alloc.tensor_shape), mybir.dt.np(alloc.dtype)))
    n_params = len(in_names)
    all_in_names = list(in_names) + list(out_names)
    if partition_name is not None:
        all_in_names.append(partition_name)
    donate = tuple(range(n_params, n_params + len(out_names)))

    def _body(*args):
        operands = list(args)
        if partition_name is not None:
            operands.append(bass2jax.partition_id_tensor())
        return tuple(bass2jax._bass_exec_p.bind(
            *operands, out_avals=tuple(out_avals), in_names=tuple(all_in_names),
            out_names=tuple(out_names), lowering_input_output_aliases=(),
            sim_require_finite=True, sim_require_nnan=True, nc=nc))

    devices = jax.devices()[:N_CORES]
    mesh = Mesh(np.asarray(devices), ("core",))
    sharded = jax.jit(
        shard_map(_body, mesh=mesh,
                  in_specs=(PartitionSpec("core"),) * (n_params + len(out_names)),
                  out_specs=(PartitionSpec("core"),) * len(out_names),
                  check_rep=False),
        donate_argnums=donate, keep_unused=True)

    def run(in_maps):
        concat_in = [np.concatenate([np.asarray(m[nm]) for m in in_maps], axis=0)
                     for nm in in_names]
        zeros = [np.zeros((N_CORES * a.shape[0], *a.shape[1:]),
                          np.dtype(a.dtype)) for a in out_avals]
        outs = sharded(*concat_in, *zeros)
        out0 = np.asarray(outs[0])
        return out0.reshape(N_CORES, *out_avals[0].shape)

    _CACHE["run"] = run
    return run


def _make_in_maps(inputs) -> list:
    gnn_in = np.ascontiguousarray(np.asarray(inputs["gnn_in"], dtype=np.float32))
    centers = np.ascontiguousarray(np.asarray(inputs["centers"], dtype=np.float32))

    # node weights: 8 blocks [D, 128] f16 -> bitcast [D, 512] f32
    blocks = []
    for li in (1, 2):
        Wf = np.asarray(inputs[f"Wf{li}"], np.float32)
        Ws = np.asarray(inputs[f"Ws{li}"], np.float32)
        blocks += [Wf[0:D], Wf[D:2 * D], Ws[0:D], Ws[D:2 * D]]
    wn16 = np.ascontiguousarray(np.concatenate(blocks, axis=1)).astype(np.float16)
    wn32 = wn16.view(np.float32)                      # [D, 512]

    biases = np.stack([np.asarray(inputs[nm], np.float32)
                       for nm in ("bf1", "bs1", "gamma1", "beta1",
                                  "bf2", "bs2", "gamma2", "beta2")], axis=1)  # [D, 8]

    # edge weights: 8 blocks [2, 128] f16
    eblocks = []
    for li in (1, 2):
        Wf = np.asarray(inputs[f"Wf{li}"], np.float32)
        Ws = np.asarray(inputs[f"Ws{li}"], np.float32)
        eblocks += [Wf[2 * D:], -Wf[2 * D:], Ws[2 * D:], -Ws[2 * D:]]
    ew16 = np.ascontiguousarray(np.concatenate(eblocks, axis=1)).astype(np.float16)  # [2, 1024]

    in_maps = []
    for c in range(N_CORES):
        sl = slice(c * NODES_PC, (c + 1) * NODES_PC)
        ct16 = np.ascontiguousarray(centers[sl].T).astype(np.float16)  # [2, 1024]
        ec16 = np.concatenate([ct16, ew16], axis=1)   # [2, 2048]
        ec32 = np.ascontiguousarray(ec16).reshape(D, EC_COLS * 2).view(np.float32)
        xt = np.zeros((D, TOTC), np.float32)
        xt[:, 0:XCOLS] = gnn_in[sl].T
        xt[:, BIAS0:BIAS0 + 8] = biases
        xt[:, WN0:WN0 + 8 * WN_BLK] = wn32
        xt[:, EC0:IDEN0] = ec32
        xt[0:AGENTS, IDEN0:TOTC] = np.eye(AGENTS, dtype=np.float16).view(np.float32)
        in_maps.append({"xt": xt})
    return in_maps


# revision 28
# speedup vs baseline: 7.5786x; 7.5786x over previous
"""Trainium2 Bass kernel for nn_AgentGnn (2-layer CGConv GNN, 128 scenes x 64 agents).

Structure exploited:
- Edges are fully-connected per 64-agent scene (no self loops), so gather/scatter
  becomes dense 64x64 blocks: agg[i] = sum_j sigmoid(F_ij) * softplus(S_ij) - diag.
- Per-edge linear terms factor into per-node terms:
    F_ij = af[i] + bf[j],  af = x_i @ Wf[:D] + c_i @ Wf[2D:] (+bias via ACT),
                           bf = x_j @ Wf[D:2D] - c_j @ Wf[2D:]
- Pairwise sums F[d,(i,j)] are built on TensorE with a 0/1 indicator matmul
  (indicator generated on device via affine_select) against a stacked
  [af_scene; bf_scene] stationary operand.
- softplus = ln(1+exp(.)) (Exp+Ln share one ACT table set); sigmoid in another
  set; all 16 scenes batched per set so tables load twice per layer.
- BatchNorm stats are global over all 8192 nodes -> tiny [128,2] AllReduce/layer,
  computed in two fused reductions over the full agg tile at layer end.
- All inputs packed into ONE [128, 1560] f32 tensor per core (x | biases |
  bitcast-f16 node weights | packed centers+edge weights) to minimize
  per-dispatch argument overhead. Sharding: 16 scenes per core, data parallel.
"""

import numpy as np

N_SAMPLES = 128
AGENTS = 64
D = 128
EDIM = 2
N = N_SAMPLES * AGENTS
EPS = 1e-5

N_CORES = 8
SCENES_PC = N_SAMPLES // N_CORES      # 16 scenes per core
NODES_PC = SCENES_PC * AGENTS         # 1024 nodes per core
PAIR = AGENTS * AGENTS                # 4096 pairwise cols per scene
CHUNK = 2048                          # pairwise chunk (32 i x 64 j)
N_CHUNKS = PAIR // CHUNK              # 2
I_PER_CHUNK = CHUNK // AGENTS         # 32
SUB = 8                               # scenes per sigmoid sub-batch

# packed input column layout (f32 columns)
XCOLS = NODES_PC          # 0:1024      x.T (f32)
BIAS0 = XCOLS             # 1024:1032   bf1,bs1,ga1,be1,bf2,bs2,ga2,be2
WN0 = BIAS0 + 8           # 1032:1544   node weights, f16 bitcast (8 x [D,128])
WN_BLK = D // 2           # 64 f32 cols per [D,128] f16 block
EC0 = WN0 + 8 * WN_BLK    # 1544:1560   centers.T + edge weights, f16 packed
EC_F16 = EDIM * (NODES_PC + 8 * D)    # 2 x (1024 + 1024) f16 values
EC_COLS = EC_F16 // 2 // D            # 16 f32 cols
IDEN0 = EC0 + EC_COLS     # 1560:1592   identity64 f16 seed (rows 0:64)
TOTC = IDEN0 + 32         # 1592

_CACHE: dict = {}


def _expected_edges():
    a = np.arange(AGENTS)
    rows = np.repeat(a, AGENTS)
    cols = np.tile(a, AGENTS)
    mask = rows != cols
    rows, cols = rows[mask], cols[mask]
    offsets = (np.arange(N_SAMPLES) * AGENTS)[:, None]
    src = (rows[None, :] + offsets).ravel().astype(np.int32)
    dst = (cols[None, :] + offsets).ravel().astype(np.int32)
    return src, dst


def _numpy_fallback(gnn_in, centers, edge_src, edge_dst, ws):
    # generic (slow) reference path, used only if edges don't match the
    # expected block-diagonal fully-connected pattern
    def sigmoid(x):
        return 1.0 / (1.0 + np.exp(-x))

    def softplus(x):
        return np.logaddexp(0.0, x)

    x = gnn_in.astype(np.float64)
    e = (centers[edge_dst] - centers[edge_src]).astype(np.float64)
    for li in (1, 2):
        Wf, bf, Ws, bs, gamma, beta = (ws[f"Wf{li}"], ws[f"bf{li}"], ws[f"Ws{li}"],
                                       ws[f"bs{li}"], ws[f"gamma{li}"], ws[f"beta{li}"])
        z = np.concatenate([x[edge_dst], x[edge_src], e], axis=-1)
        m = sigmoid(z @ Wf + bf) * softplus(z @ Ws + bs)
        agg = np.zeros((N, D))
        np.add.at(agg, edge_dst, m)
        mu = agg.mean(axis=0)
        var = agg.var(axis=0)
        agg = (agg - mu) / np.sqrt(var + EPS) * gamma + beta
        x = np.maximum(agg + x, 0.0)
    return x.astype(np.float32)


def _build_nc(use_collectives=True, unroll=1, dup_act=1, dup_vec=1, dup_mm=1, old_tree=False, exp_sbuf=True, scene_tree=False):
    import concourse.bacc as bacc
    import concourse.mybir as mybir
    import concourse.tile as tile
    from concourse.tile_rust import add_dep_helper

    f32 = mybir.dt.float32
    f16 = mybir.dt.float16
    bf16 = mybir.dt.bfloat16
    AF = mybir.ActivationFunctionType
    OP = mybir.AluOpType

    nc = bacc.Bacc("TRN2", target_bir_lowering=False, debug=False,
                   num_devices=N_CORES if use_collectives else 1)

    # ---- I/O: single packed input ----
    xt_in = nc.dram_tensor("xt", [D, TOTC], f32, kind="ExternalInput")
    out_t = nc.dram_tensor("out_t", [D, NODES_PC], f32, kind="ExternalOutput")

    acts = []  # ACT instructions in intended engine order

    def act(*args, **kwargs):
        for _ in range(dup_act):
            inst = nc.scalar.activation(*args, **kwargs)
            acts.append(inst)
        return inst

    def vtt(*args, **kwargs):
        for _ in range(dup_vec):
            inst = nc.vector.tensor_tensor(*args, **kwargs)
        return inst

    def mm(*args, **kwargs):
        for _ in range(dup_mm):
            inst = nc.tensor.matmul(*args, **kwargs)
        return inst

    with tile.TileContext(nc) as tc:
        with (
            tc.tile_pool(name="cst", bufs=1) as cst,
            tc.tile_pool(name="wrk", bufs=1) as wrk,
            tc.tile_pool(name="chk", bufs=3) as chk,
            tc.tile_pool(name="epool", bufs=4) as epool,
            tc.tile_pool(name="ps", bufs=2, space="PSUM") as ps,
            tc.tile_pool(name="dram", bufs=1, space="DRAM") as dram,
        ):
            # ---- load + unpack the single input ----
            xt = cst.tile([D, TOTC], f32)
            nc.sync.dma_start(xt[:], xt_in.ap())
            ec = cst.tile([EDIM, EC_F16 // EDIM], f16)   # [2, 2048]
            nc.sync.dma_start(ec[:], xt[:, EC0:IDEN0].bitcast(f16))

            wn16 = cst.tile([D, 8 * D], f16)
            nc.vector.tensor_copy(
                wn16[:], xt[:, WN0:WN0 + 8 * WN_BLK].bitcast(f16))

            def wblk(idx):
                return wn16[:, idx * D:(idx + 1) * D]      # [D, 128] f16

            def eblk(idx):
                c0 = NODES_PC + idx * D
                return ec[:, c0:c0 + D]                    # [2, 128] f16

            def bias(idx):
                return xt[:, BIAS0 + idx:BIAS0 + idx + 1]  # [D, 1] f32

            # ---- indicator built on device from the identity seed ----
            iden = cst.tile([AGENTS, AGENTS], f16)
            nc.sync.dma_start(iden[:], xt[0:AGENTS, IDEN0:TOTC].bitcast(f16))
            ind = cst.tile([128, PAIR], f16)
            nc.vector.tensor_copy(
                ind[0:64, :].rearrange("p (a b) -> p a b", b=AGENTS),
                iden[:].rearrange("p (a b) -> p a b", b=1).broadcast_to(
                    [AGENTS, AGENTS, AGENTS]))
            nc.vector.tensor_copy(
                ind[64:128, :].rearrange("p (a b) -> p a b", b=AGENTS),
                iden[:].rearrange("p (a b) -> p a b", a=1).broadcast_to(
                    [AGENTS, AGENTS, AGENTS]))

            x_cur = xt[:, 0:NODES_PC]
            for rep in range(unroll):
              for li0 in (1, 2):
                li = f"{li0}" if rep == 0 else f"{li0}r{rep}"
                w_off = (li0 - 1) * 4
                wfd, wfs = wblk(w_off + 0), wblk(w_off + 1)
                wsd, wss = wblk(w_off + 2), wblk(w_off + 3)
                wfe, wfen = eblk(w_off + 0), eblk(w_off + 1)
                wse, wsen = eblk(w_off + 2), eblk(w_off + 3)
                bf, bs = bias((li0 - 1) * 4 + 0), bias((li0 - 1) * 4 + 1)
                ga, be = bias((li0 - 1) * 4 + 2), bias((li0 - 1) * 4 + 3)

                # fp16 copy of x for the node matmuls (residual stays fp32)
                x16 = wrk.tile([D, NODES_PC], f16, name=f"x16_{li}", tag="x16")
                nc.vector.tensor_copy(x16[:], x_cur)

                agg = wrk.tile([D, NODES_PC], f32, name=f"agg{li}", tag="agg")

                abf, abs_ = {}, {}

                def node_phase(batch, bi):
                    # one PSUM slab per batch: F-path node terms [af;bf] in
                    # cols 0:1024 (node-major, feeds the indicator matmul) and
                    # S-path terms TRANSPOSED (d-major: as 1024:1536, bs
                    # 1536:2048) so exp factorizes per node and two batched
                    # Exp ACTs replace the full-size pairwise Exp pass.
                    nb = len(batch)
                    slab = ps.tile([128, 4 * AGENTS * nb], f32,
                                   name=f"slab{bi}_{li}", tag="pp")
                    a0 = 2 * AGENTS * nb          # as-region start (1024)
                    b0 = 3 * AGENTS * nb          # bs-region start (1536)
                    for k, s in enumerate(batch):
                        xs = x16[:, s * AGENTS:(s + 1) * AGENTS]
                        cs = ec[:, s * AGENTS:(s + 1) * AGENTS]
                        pab = slab[:, k * D:(k + 1) * D]
                        nc.tensor.matmul(pab[0:64, :], lhsT=cs, rhs=wfe, start=True, stop=False)
                        nc.tensor.matmul(pab[0:64, :], lhsT=xs, rhs=wfd, start=False, stop=True)
                        nc.tensor.matmul(pab[64:128, :], lhsT=cs, rhs=wfen, start=True, stop=False)
                        nc.tensor.matmul(pab[64:128, :], lhsT=xs, rhs=wfs, start=False, stop=True)
                        pas = slab[:, a0 + k * AGENTS:a0 + (k + 1) * AGENTS]
                        nc.tensor.matmul(pas, lhsT=wse, rhs=cs, start=True, stop=False)
                        nc.tensor.matmul(pas, lhsT=wsd, rhs=xs, start=False, stop=True)
                        pbs = slab[:, b0 + k * AGENTS:b0 + (k + 1) * AGENTS]
                        nc.tensor.matmul(pbs, lhsT=wsen, rhs=cs, start=True, stop=False)
                        nc.tensor.matmul(pbs, lhsT=wss, rhs=xs, start=False, stop=True)
                    abt = wrk.tile([128, 2 * AGENTS * nb], f16,
                                   name=f"abt{bi}_{li}", tag=f"abt{bi}")
                    nc.vector.tensor_copy(abt[:], slab[:, 0:a0])
                    es = wrk.tile([D, 2 * AGENTS * nb], bf16,
                                  name=f"es{bi}_{li}", tag=f"es{bi}")
                    act(es[:, 0:AGENTS * nb], slab[:, a0:b0], AF.Exp,
                        bias=bs, scale=1.0)
                    act(es[:, AGENTS * nb:], slab[:, b0:], AF.Exp,
                        bias=0.0, scale=1.0)
                    for k, s in enumerate(batch):
                        abf[s] = abt[:, k * D:(k + 1) * D]
                        abs_[s] = (es[:, k * AGENTS:(k + 1) * AGENTS],
                                   es[:, (nb + k) * AGENTS:(nb + k + 1) * AGENTS])

                def phase_a(batch, sgs, Es):
                    for s in batch:
                        # prebuild E = ea (x) eb on DVE during the sigma window
                        # (DVE is otherwise idle here; keeps phase_b's Ln ACTs
                        # from stalling behind the outer products)
                        ea, eb = abs_[s]
                        E = epool.tile([D, PAIR], bf16, name=f"E{s % 4}", tag="E")
                        Es[s] = E
                        vtt(E[:].rearrange("p (a b) -> p a b", b=AGENTS),
                            ea.rearrange("p (a b) -> p a b", b=1).broadcast_to(
                                [D, AGENTS, AGENTS]),
                            eb.rearrange("p (a b) -> p a b", a=1).broadcast_to(
                                [D, AGENTS, AGENTS]),
                            OP.mult)
                        sg = wrk.tile([D, PAIR], f16, name=f"sg{s % SUB}",
                                      tag=f"sg{s % SUB}")
                        sgs[s] = sg
                        for c in range(N_CHUNKS):
                            pf = ps.tile([D, CHUNK], f32, name=f"pf{s}_{c}", tag="pp")
                            for k in range(CHUNK // 512):
                                col = c * CHUNK + k * 512
                                mm(pf[:, k * 512:(k + 1) * 512],
                                   lhsT=abf[s],
                                   rhs=ind[:, col:col + 512],
                                   start=True, stop=True)
                            act(sg[:, c * CHUNK:(c + 1) * CHUNK], pf[:],
                                AF.Sigmoid, bias=bf, scale=1.0)
                        # zero sigma on the diagonal -> m_ii = 0, so the
                        # j-reduction needs no diagonal correction
                        nc.vector.memset(
                            sg[:, 0:(AGENTS - 1) * (AGENTS + 1) + 1:AGENTS + 1],
                            0.0)

                def phase_b(batch, sgs, Es):
                    for s in batch:
                        E = Es[s]
                        pch = chk.tile([D, PAIR], f16, name="pch", tag="pch")
                        act(pch[:], E[:], AF.Ln, bias=1.0, scale=1.0)
                        uch = chk.tile([D, PAIR], f16, name="uch", tag="uch")
                        vtt(uch[:], sgs[s][:], pch[:], OP.mult)
                        for c in range(N_CHUNKS):
                            cur = uch[:, c * CHUNK:(c + 1) * CHUNK]
                            w = AGENTS
                            while w > 16:
                                h = w // 2
                                c3 = cur.rearrange("p (i j) -> p i j", j=w)
                                nxt = chk.tile([D, I_PER_CHUNK * h], f16,
                                               name=f"trc{h}", tag=f"trc{h}")
                                n3 = nxt.rearrange("p (i j) -> p i j", j=h)
                                vtt(n3, c3[:, :, 0:h], c3[:, :, h:w], OP.add)
                                cur = nxt[:]
                                w = h
                            off = s * AGENTS + c * I_PER_CHUNK
                            nc.vector.tensor_reduce(
                                agg[:, off:off + I_PER_CHUNK],
                                cur.rearrange("p (i j) -> p i j", j=w),
                                axis=mybir.AxisListType.X, op=OP.add)

                batches = [list(range(b0, min(b0 + SUB, SCENES_PC)))
                           for b0 in range(0, SCENES_PC, SUB)]
                sgs_all = [dict() for _ in batches]
                es_all = [dict() for _ in batches]
                node_phase(batches[0], 0)
                for bi, batch in enumerate(batches):
                    phase_a(batch, sgs_all[bi], es_all[bi])
                    if bi + 1 < len(batches):
                        node_phase(batches[bi + 1], bi + 1)
                    phase_b(batch, sgs_all[bi], es_all[bi])

                # ---- BN stats over the full agg tile (sum, sumsq) ----
                stats = wrk.tile([D, 2], f32, name=f"stats{li}", tag="stats")
                sq = wrk.tile([D, NODES_PC], f32, name="sq", tag="sg0")
                nc.vector.tensor_reduce(stats[:, 0:1], agg[:],
                                        axis=mybir.AxisListType.X, op=OP.add)
                nc.vector.tensor_tensor(sq[:], agg[:], agg[:], OP.mult)
                nc.vector.tensor_reduce(stats[:, 1:2], sq[:],
                                        axis=mybir.AxisListType.X, op=OP.add)

                # ---- AllReduce stats across cores ----
                cc_in = dram.tile([D, 2], f32, name=f"ccin{li}", tag=f"ccin{li}")
                cc_out = dram.tile([D, 2], f32, name=f"ccout{li}", tag=f"ccout{li}",
                                   addr_space="Shared")
                nc.sync.dma_start(cc_in[:], stats[:])
                if use_collectives:
                    nc.gpsimd.collective_compute(
                        "AllReduce", OP.add,
                        replica_groups=[list(range(N_CORES))],
                        ins=[cc_in.opt()], outs=[cc_out.opt()])
                else:
                    nc.sync.dma_start(cc_out[:], cc_in[:])
                stot = wrk.tile([D, 2], f32, name=f"stot{li}", tag="stot")
                nc.sync.dma_start(stot[:], cc_out[:])

                # mu, var, rstd = exp(-0.5*ln(var+eps)); A = gamma*rstd; B = beta-mu*A
                mu = wrk.tile([D, 1], f32, name="mu", tag="mu")
                ex2 = wrk.tile([D, 1], f32, name="ex2", tag="ex2")
                nc.vector.tensor_scalar_mul(mu[:], stot[:, 0:1], 1.0 / N)
                nc.vector.tensor_scalar_mul(ex2[:], stot[:, 1:2], 1.0 / N)
                var = wrk.tile([D, 1], f32, name="var", tag="var")
                nc.vector.tensor_tensor(var[:], mu[:], mu[:], OP.mult)
                nc.vector.tensor_tensor(var[:], ex2[:], var[:], OP.subtract)
                rstd = wrk.tile([D, 1], f32, name="rstd", tag="rstd")
                nc.vector.tensor_scalar_add(var[:], var[:], EPS)
                act(rstd[:], var[:], AF.Ln, bias=0.0, scale=1.0)
                act(rstd[:], rstd[:], AF.Exp, bias=0.0, scale=-0.5)
                A = wrk.tile([D, 1], f32, name="A", tag="A")
                Bt = wrk.tile([D, 1], f32, name="Bt", tag="Bt")
                nc.vector.tensor_tensor(A[:], ga, rstd[:], OP.mult)
                nc.vector.tensor_tensor(Bt[:], mu[:], A[:], OP.mult)
                nc.vector.tensor_tensor(Bt[:], be, Bt[:], OP.subtract)

                # x_next = relu(agg*A + B + x_cur), fused as STT + TS
                xn = wrk.tile([D, NODES_PC], f32, name=f"x{li}", tag=f"x{li0}")
                nc.vector.scalar_tensor_tensor(xn[:], agg[:], A[:, 0:1], x_cur,
                                               OP.mult, OP.add)
                nc.vector.tensor_scalar(xn[:], xn[:], Bt[:, 0:1], 0.0,
                                        OP.add, OP.max)
                x_cur = xn[:]

            nc.sync.dma_start(out_t.ap(), x_cur)

        # enforce ACT program order so table loads stay batched
        for a, b in zip(acts, acts[1:]):
            add_dep_helper(b.ins, a.ins, sync=False,
                           reason="ACT table-set batching order")

    # Restrict the act-table chooser to the two sets we actually want so
    # Exp and Ln resolve to the shared natural_log_exp set (the default
    # chooser alternates exp_and_others / natural_log, thrashing ~2.7us
    # table loads between every Exp and Ln).
    keep = {"sigmoid_and_others", "natural_log_exp_and_others"}
    orig_tables = bacc.get_activation_tables

    def patched_tables(arch):
        return {k: (v if k in keep else set())
                for k, v in orig_tables(arch).items()}

    bacc.get_activation_tables = patched_tables
    try:
        nc.compile()
    finally:
        bacc.get_activation_tables = orig_tables
    return nc


def _get_nc():
    if "nc" not in _CACHE:
        _CACHE["nc"] = _build_nc()
    return _CACHE["nc"]


def kernel(**inputs) -> np.ndarray:
    gnn_in = np.ascontiguousarray(np.asarray(inputs["gnn_in"], dtype=np.float32))
    centers = np.ascontiguousarray(np.asarray(inputs["centers"], dtype=np.float32))
    edge_src = np.asarray(inputs["edge_src"], dtype=np.int32)
    edge_dst = np.asarray(inputs["edge_dst"], dtype=np.int32)

    exp_src, exp_dst = _expected_edges()
    if not (np.array_equal(edge_src, exp_src) and np.array_equal(edge_dst, exp_dst)):
        return _numpy_fallback(
            gnn_in, centers, edge_src, edge_dst,
            {k: np.asarray(v, np.float32) for k, v in inputs.items()
             if k not in ("gnn_in", "centers", "edge_src", "edge_dst")})

    in_maps = _make_in_maps(inputs)
    run = _get_runner()
    per_core = run(in_maps)                      # [N_CORES, D, NODES_PC]
    out = np.concatenate(list(per_core), axis=1)  # [D, N]
    return np.ascontiguousarray(out.T)


def _get_runner():
    """Compile once; repeat kernel() calls reuse the jitted executable."""
    if "run" in _CACHE:
        return _CACHE["run"]
    import jax
    from jax.experimental.shard_map import shard_map
    from jax.sharding import Mesh, PartitionSpec

    import concourse.mybir as mybir
    from concourse import bass2jax

    bass2jax.install_neuronx_cc_hook()
    nc = _get_nc()
    partition_name = nc.partition_id_tensor.name if nc.partition_id_tensor else None
    in_names, out_names, out_avals = [], [], []
    for alloc in nc.m.functions[0].allocations:
        if not isinstance(alloc, mybir.MemoryLocationSet):
            continue
        name = alloc.memorylocations[0].name
        if alloc.kind == "ExternalInput":
            if name != partition_name:
                in_names.append(name)
        elif alloc.kind == "ExternalOutput":
            out_names.append(name)
            out_avals.append(jax.core.ShapedArray(
                tuple(alloc.tensor_shape), mybir.dt.np(alloc.dtype)))
    n_params = len(in_names)
    all_in_names = list(in_names) + list(out_names)
    if partition_name is not None:
        all_in_names.append(partition_name)
    donate = tuple(range(n_params, n_params + len(out_names)))

    def _body(*args):
        operands = list(args)
        if partition_name is not None:
            operands.append(bass2jax.partition_id_tensor())
        return tuple(bass2jax._bass_exec_p.bind(
            *operands, out_avals=tuple(out_avals), in_names=tuple(all_in_names),
            out_names=tuple(out_names), lowering_input_output_aliases=(),
            sim_require_finite=True, sim_require_nnan=True, nc=nc))

    devices = jax.devices()[:N_CORES]
    mesh = Mesh(np.asarray(devices), ("core",))
    sharded = jax.jit(
        shard_map(_body, mesh=mesh,
                  in_specs=(PartitionSpec("core"),) * (n_params + len(out_names)),
                  out_specs=(PartitionSpec("core"),) * len(out_names),
                  check_rep=False),
        donate_argnums=donate, keep_unused=True)

    def run(in_maps):
        concat_in = [np.concatenate([np.asarray(m[nm]) for m in in_maps], axis=0)
                     for nm in in_names]
        zeros = [np.zeros((N_CORES * a.shape[0], *a.shape[1:]),
                          np.dtype(a.dtype)) for a in out_avals]
        outs = sharded(*concat_in, *zeros)
        out0 = np.asarray(outs[0])
        return out0.reshape(N_CORES, *out_avals[0].shape)

    _CACHE["run"] = run
    return run


def _make_in_maps(inputs) -> list:
    gnn_in = np.ascontiguousarray(np.asarray(inputs["gnn_in"], dtype=np.float32))
    centers = np.ascontiguousarray(np.asarray(inputs["centers"], dtype=np.float32))

    # node weights: 8 blocks [D, 128] f16 -> bitcast [D, 512] f32
    blocks = []
    for li in (1, 2):
        Wf = np.asarray(inputs[f"Wf{li}"], np.float32)
        Ws = np.asarray(inputs[f"Ws{li}"], np.float32)
        blocks += [Wf[0:D], Wf[D:2 * D], Ws[0:D], Ws[D:2 * D]]
    wn16 = np.ascontiguousarray(np.concatenate(blocks, axis=1)).astype(np.float16)
    wn32 = wn16.view(np.float32)                      # [D, 512]

    biases = np.stack([np.asarray(inputs[nm], np.float32)
                       for nm in ("bf1", "bs1", "gamma1", "beta1",
                                  "bf2", "bs2", "gamma2", "beta2")], axis=1)  # [D, 8]

    # edge weights: 8 blocks [2, 128] f16
    eblocks = []
    for li in (1, 2):
        Wf = np.asarray(inputs[f"Wf{li}"], np.float32)
        Ws = np.asarray(inputs[f"Ws{li}"], np.float32)
        eblocks += [Wf[2 * D:], -Wf[2 * D:], Ws[2 * D:], -Ws[2 * D:]]
    ew16 = np.ascontiguousarray(np.concatenate(eblocks, axis=1)).astype(np.float16)  # [2, 1024]

    in_maps = []
    for c in range(N_CORES):
        sl = slice(c * NODES_PC, (c + 1) * NODES_PC)
        ct16 = np.ascontiguousarray(centers[sl].T).astype(np.float16)  # [2, 1024]
        ec16 = np.concatenate([ct16, ew16], axis=1)   # [2, 2048]
        ec32 = np.ascontiguousarray(ec16).reshape(D, EC_COLS * 2).view(np.float32)
        xt = np.zeros((D, TOTC), np.float32)
        xt[:, 0:XCOLS] = gnn_in[sl].T
        xt[:, BIAS0:BIAS0 + 8] = biases
        xt[:, WN0:WN0 + 8 * WN_BLK] = wn32
        xt[:, EC0:IDEN0] = ec32
        xt[0:AGENTS, IDEN0:TOTC] = np.eye(AGENTS, dtype=np.float16).view(np.float32)
        in_maps.append({"xt": xt})
    return in_maps
